# revision 83
# baseline (speedup 1.0000x reference)
"""Trainium2 Bass kernel for nn_AblationAttention (sliding-window causal
attention, W=256, with per-head RMSNorm on q/k).

Key math fact: the reference's "genetic fitness" block adds log(fitness)[b,h,q]
to scores — a constant along the softmax (k) axis — so softmax is invariant to
it and the block is a no-op for the output. We compute plain sliding-window
causal attention.

Sharding: 8 cores = batch (2) x head-group (4 groups of 4 heads).
Each core: full x for its batch (pre-transposed on host), column-sliced
wq/wk/wv, row-sliced wo. Host sums the 4 head-group partials per batch.

v6: cost-model-tuned pipeline. PSUM banks: qk(2) v(1) o(1) tps(1) s(2) c(1).
Every producer->consumer pair gets >= 1 full round of slack (stats chain for
tile m spans rounds m..m+1, aT at m+2, scores kt at r-5, o at r-6, oT at r-7,
outproj at r-9) so the list scheduler never hits a zero-slack cross-engine
hop; the last 3 tiles of each stage run with tighter dependency-limited lags
since the engines are idle by then, and their outproj halves borrow the idle
qk banks so they don't serialize through the single c bank. v-projection lags
qk by two rounds so wv loads late. Per-round engine budget: PE ~4.7us (the
bound), ACT ~3.6us, DVE ~3.4us, Pool ~1.8us. Evacuation copies are merged and
balanced across ACT/DVE; affine selects process both heads of a pair in one
op; DMAs use >=512B descriptors; a junk-transpose warmup holds the PE p-state
ramp at full clock before the first projections.

Phase B is k-major: scores are computed transposed (sT[k,q]) one key-tile at
a time against the <=3 query-tiles whose sliding window contains it, so exp
emits p^T directly in the layout the o-matmul wants. The causal/window mask
is applied by zeroing invalid probabilities post-exp with Pool-engine
affine_selects. A ones-column appended to each V block makes the o-matmul
accumulate the softmax denominator as column 64 of each head block.

rmsnorm's rsqrt is computed as exp(-0.5*ln(x)) so every activation used
(square/ln/exp/copy) lives in one ACT table.
"""

import sys

sys.path.insert(0, "/opt/trn_rl_repo")

import numpy as np
import ml_dtypes

import concourse.bass as bass
import concourse.tile as tile
from concourse import bacc, mybir
from concourse import bass_utils
from concourse.masks import make_identity
from concourse.hw_specs import get_activation_tables

# Problem constants (hardcoded per harness contract)
B, T, E, H, W = 2, 2048, 1024, 16, 256
D = E // H  # 64
NCORES = 8
HG = 4  # head-groups
HPG = H // HG  # heads per core = 4
COLS = HPG * D  # 256
VC = D + 1  # v block width incl ones column = 65
EPS = float(np.finfo(np.float32).eps)
FP = mybir.dt.float32
BF = mybir.dt.bfloat16
NT = T // 128  # 16 tiles
AF = mybir.ActivationFunctionType
ALU = mybir.AluOpType

_cache = {}


def _ap(t, extra_off, dims):
    """Custom AP on tile view t: partition dim kept, free dims replaced."""
    return bass.AP(tensor=t.tensor, offset=t.offset + extra_off, ap=[t.ap[0]] + dims)


def _build():
    nc = bacc.Bacc(
        "TRN2",
        target_bir_lowering=False,
        debug=False,
        enable_asserts=False,
        num_devices=NCORES,
    )
    xT = nc.dram_tensor("xT", [E, T], BF, kind="ExternalInput").ap()
    wqk = nc.dram_tensor("wqk", [E, 2 * COLS], BF, kind="ExternalInput").ap()
    wv = nc.dram_tensor("wv", [E, COLS], BF, kind="ExternalInput").ap()
    wo = nc.dram_tensor("wo", [COLS, E], BF, kind="ExternalInput").ap()
    qn2 = nc.dram_tensor("qn2", [128, 1], FP, kind="ExternalInput").ap()
    kn2 = nc.dram_tensor("kn2", [128, 1], FP, kind="ExternalInput").ap()
    out = nc.dram_tensor("out", [T, E], BF, kind="ExternalOutput").ap()

    with tile.TileContext(nc) as tc:
        with (
            tc.tile_pool(name="singles", bufs=1) as singles,
            tc.tile_pool(name="xin", bufs=4) as xin,
            tc.tile_pool(name="work", bufs=6) as work,
            tc.tile_pool(name="stats", bufs=8) as stats,
            tc.tile_pool(name="outst", bufs=6) as outst,
            tc.tile_pool(name="ps", bufs=1, space="PSUM") as ps,
        ):
            # one ACT table covers square/ln/exp/copy — load it explicitly so
            # the table-load pass doesn't greedily thrash between smaller sets
            set_id = list(get_activation_tables(nc.m.arch)).index(
                "natural_log_exp_and_others"
            )
            nc.scalar.add_instruction(
                mybir.InstLoadActFuncSet(
                    name=f"I-{nc.next_id()}", act_func_set_id=set_id,
                    engine=mybir.EngineType.Activation,
                )
            )

            # ---- resident tensors ----
            wqk_sb = singles.tile([128, 8, 2 * COLS], BF, tag="wqk")
            wv_sb = singles.tile([128, 8, COLS], BF, tag="wv")
            wo_sb = singles.tile([128, 2, E], BF, tag="wo")
            qn_sb = singles.tile([128, 1], FP, tag="qn2")
            kn_sb = singles.tile([128, 1], FP, tag="kn2")
            idb_sb = singles.tile([128, 128], BF, tag="identb")
            eps_sb = singles.tile([128, 1], FP, tag="eps")
            # combined q/k transposed: c=0,1 -> q head-pairs, c=2,3 -> k
            qkT_sb = singles.tile([128, 4, T], BF, tag="qkT")
            # v blocks with a ones column per head: [t, kt, 4*(64 v | 1)]
            v_sb = singles.tile([128, NT, HPG * VC], BF, tag="vsb")
            hoT_sb = [
                singles.tile([128, 2, 512], BF, tag=f"hoT{g}", name=f"hoT{g}")
                for g in range(4)
            ]

            xT_r = xT.rearrange("(k p) t -> p k t", p=128)
            xmap = {}

            def load_x(tc_i):
                # one big dma per chunk: 512 t-cols => 1KB rows, no RMW penalty
                xmap[tc_i] = xin.tile([128, 8, 512], BF, tag="xT", name="x_t")
                t0 = tc_i * 512
                nc.sync.dma_start(
                    out=xmap[tc_i], in_=xT_r[:, :, t0 : t0 + 512]
                )

            # startup: x chunk 0 as two independent tiles (so projections of
            # m=0,1 never serialize against the second half's DMA), wqk
            # per-kc so the first qk matmuls start as slices land
            xmap["0a"] = xin.tile([128, 8, 256], BF, tag="xT0", name="x_0a", bufs=4)
            xmap["0b"] = xin.tile([128, 8, 256], BF, tag="xT0", name="x_0b", bufs=4)
            xmap["1a"] = xin.tile([128, 8, 256], BF, tag="xT0", name="x_1a", bufs=4)
            xmap["1b"] = xin.tile([128, 8, 256], BF, tag="xT0", name="x_1b", bufs=4)
            nc.sync.dma_start(out=xmap["0a"], in_=xT_r[:, :, 0:256])
            wqk_r = wqk.rearrange("(k p) c -> p k c", p=128)
            for kc in range(8):
                nc.sync.dma_start(
                    out=wqk_sb[:, kc : kc + 1, :], in_=wqk_r[:, kc : kc + 1, :]
                )
            nc.sync.dma_start(out=qn_sb, in_=qn2)
            nc.sync.dma_start(out=kn_sb, in_=kn2)
            nc.sync.dma_start(out=xmap["0b"], in_=xT_r[:, :, 256:512])
            nc.sync.dma_start(
                out=wv_sb, in_=wv.rearrange("(k p) c -> p k c", p=128)
            )
            # warm the PE p-state ramp with junk matmuls on a zeroed scratch
            # tile while the first x/wqk DMAs are in flight, so the first
            # projections run at full clock. The junk tile is the first tps
            # alloc; the real first user (aT(0), ~12us in) absorbs the WAW.
            junk_sb = singles.tile([128, 128], BF, tag="junk")
            nc.vector.memset(junk_sb, 0.0)
            warm = ps.tile([128, 768], BF, tag="tps", bufs=1, name="warm")
            for i in range(30):
                nc.tensor.transpose(
                    warm[:, (i % 6) * 128 : (i % 6 + 1) * 128], junk_sb, junk_sb
                )

            make_identity(nc, idb_sb)
            nc.vector.memset(eps_sb, EPS)
            # ones columns of v blocks: [128, NT, HPG] strided at offset 64
            nc.vector.memset(_ap(v_sb, D, [[HPG * VC, NT], [VC, HPG]]), 1.0)

            qk_map = {}     # m -> qk_ps psum tile
            sq_map = {}     # m -> (sq, ssq8, rstd8)
            nrm_map = {}    # m -> nrm sbuf tile
            tps_map = {}    # r -> tps psum tile (q/k transposes + o transpose)
            pT_map = {}     # (kt, hp) -> pT2 tile [128, 2, 384]
            o_map = {}      # qt -> o_ps
            onrm_map = {}   # qt -> o_nrm sbuf tile
            c_map = {}      # (m, half) -> c_ps
            osb_map = {}    # m -> o_sb

            def get_tps(r):
                # one [q/k-transpose | o-transpose] psum tile per round
                if tps_map.get("r") != r:
                    tps_map["r"] = r
                    tps_map["cur"] = ps.tile(
                        [128, 768], BF, tag="tps", bufs=1, name="tps"
                    )
                return tps_map["cur"]

            # ---- S1 (per head): scores matmul (PE) + exp (ACT) ------------
            def emit_b_mm(kt, hp, hi):
                nq = min(3, NT - kt)
                c0 = kt * 128
                po = slice(hi * 64, hi * 64 + 64)
                s_ps = ps.tile([128, 512], FP, tag="s_ps", bufs=2, name="s_ps")
                nc.tensor.matmul(
                    s_ps[:, : nq * 128],
                    qkT_sb[po, 2 + hp, c0 : c0 + 128],
                    qkT_sb[po, hp, c0 : c0 + nq * 128],
                    start=True, stop=True,
                )
                return s_ps

            def emit_b_exp(kt, hp, hi, s_ps):
                nq = min(3, NT - kt)
                if hi == 0:
                    pT_map[(kt, hp)] = work.tile(
                        [128, 2, 384], BF, tag="pT", name="pT", bufs=12
                    )
                pT = pT_map[(kt, hp)]
                nc.scalar.activation(
                    pT[:, hi, : nq * 128], s_ps[:, : nq * 128], AF.Exp
                )

            def emit_b_sel(kt, hp):
                # masks for both heads of the pair in one Pool op each
                nq = min(3, NT - kt)
                pT = pT_map[(kt, hp)]
                # j=0 block (qt==kt): keep where q_local >= k_local
                nc.gpsimd.affine_select(
                    pT[:, :, 0:128], pT[:, :, 0:128],
                    pattern=[[0, 2], [1, 128]], compare_op=ALU.is_ge, fill=0.0,
                    base=0, channel_multiplier=-1,
                )
                # j=2 block (qt==kt+2): keep where k_local > q_local
                if nq == 3:
                    nc.gpsimd.affine_select(
                        pT[:, :, 256:384], pT[:, :, 256:384],
                        pattern=[[0, 2], [-1, 128]], compare_op=ALU.is_ge,
                        fill=0.0, base=-1, channel_multiplier=1,
                    )

            def x_slice(m):
                if m < 8:
                    x_t = xmap[("0a", "0b", "1a", "1b")[m // 2]]
                    ml = m % 2
                else:
                    x_t, ml = xmap[m // 4], m % 4
                return x_t, slice(ml * 128, (ml + 1) * 128)

            # ---- S2: qk projection (PE) -----------------------------------
            def emit_qk_mm(m):
                x_t, sl = x_slice(m)
                # m=2 borrows the still-idle c bank so the first qk tiles are
                # not gated by the cold-start stats chains (the qk bank
                # rotation waits on nrm(m-2))
                if m == 2:
                    qk_ps = ps.tile([128, 2 * COLS], FP, tag="c_ps", bufs=1)
                else:
                    qk_ps = ps.tile([128, 2 * COLS], FP, tag="qk_ps", bufs=2)
                qk_map[m] = qk_ps
                for kc in range(8):
                    nc.tensor.matmul(qk_ps, x_t[:, kc, sl], wqk_sb[:, kc, :],
                                     start=(kc == 0), stop=(kc == 7))

            # ---- S2v: v projection, lagged (PE + DVE) ---------------------
            def emit_v_mm(m):
                x_t, sl = x_slice(m)
                v_ps = ps.tile([128, COLS], FP, tag="v_ps", bufs=1)
                for kc in range(8):
                    nc.tensor.matmul(v_ps, x_t[:, kc, sl], wv_sb[:, kc, :],
                                     start=(kc == 0), stop=(kc == 7))
                emit_v_mm.cur = v_ps

            def emit_v_copy(m):
                # strided copy into the 65-wide head blocks (bf16 cast)
                vdst = _ap(v_sb[:, m, :], 0, [[VC, HPG], [1, D]])
                nc.vector.tensor_copy(
                    vdst, emit_v_mm.cur.rearrange("p (h d) -> p h d", h=HPG)
                )

            # ---- S2b: rmsnorm stats (ACT sq -> DVE reduce -> ACT ln/exp) --
            def emit_sq(m):
                sq = work.tile([128, 2 * COLS], FP, tag="sq", bufs=3)
                nc.scalar.activation(sq, qk_map[m], AF.Square)
                sq_map[m] = sq

            def emit_reduce(m):
                sq = sq_map[m]
                ssq8 = stats.tile([128, 2 * HPG], FP, tag="ssq8", bufs=3)
                nc.vector.tensor_reduce(
                    ssq8, sq.rearrange("p (g d) -> p g d", g=2 * HPG),
                    axis=mybir.AxisListType.X, op=ALU.add,
                )
                sq_map[m] = (sq, ssq8)

            def emit_rstd(m):
                _, ssq8 = sq_map.pop(m)
                # rstd = exp(-0.5 * ln(ms + eps)) — keeps ACT in one table
                lns = stats.tile([128, 2 * HPG], FP, tag="lns", bufs=3)
                nc.scalar.activation(lns, ssq8, AF.Ln, bias=eps_sb, scale=1.0 / D)
                rstd8 = stats.tile([128, 2 * HPG], FP, tag="rstd8", bufs=3)
                nc.scalar.activation(rstd8, lns, AF.Exp, scale=-0.5)
                return rstd8

            def emit_nrm(m, rstd8):
                # evacuate qk_ps with the rstd scaling fused, q and k at once
                qk_ps = qk_map.pop(m)
                nrm = work.tile([128, 2 * COLS], BF, tag="nrm", bufs=4)
                rstd_b = bass.AP(
                    tensor=rstd8.tensor, offset=rstd8.offset,
                    ap=[rstd8.ap[0], [rstd8.ap[1][0], 2 * HPG], [0, D]],
                )
                nc.vector.tensor_mul(
                    nrm.rearrange("p (g d) -> p g d", g=2 * HPG),
                    qk_ps.rearrange("p (g d) -> p g d", g=2 * HPG),
                    rstd_b,
                )
                nrm_map[m] = nrm

            # ---- S4: transpose q/k tile m into qkT (PE) -------------------
            def emit_aT(m, r):
                nrm = nrm_map.pop(m)
                tps = get_tps(r)
                for c in range(4):
                    nc.tensor.transpose(
                        tps[:, c * 128 : (c + 1) * 128],
                        nrm[:, c * 128 : (c + 1) * 128], idb_sb,
                    )

            def emit_aT_copy(m):
                tps = tps_map["cur"]
                dq = qkT_sb[:, 0:2, m * 128 : (m + 1) * 128]
                dk = qkT_sb[:, 2:4, m * 128 : (m + 1) * 128]
                t4 = tps[:, 0:512].rearrange("p (a b) -> p a b", a=4)
                # q on DVE, k on ACT (engine balance); per-partition norm wgt
                nc.vector.tensor_scalar_mul(dq, t4[:, 0:2, :], qn_sb)
                nc.scalar.activation(dk, t4[:, 2:4, :], AF.Copy, scale=kn_sb)

            # ---- S5: o-matmuls (PE) + normalize (DVE) ---------------------
            def emit_o(qt):
                # all o-matmuls of a query tile in one contiguous blob:
                # accumulation groups in one PSUM tile must not interleave
                first = max(0, qt - 2)
                o_ps = ps.tile([128, HPG, VC], FP, tag="o_ps", bufs=1, name="o_ps")
                o_map[qt] = o_ps
                for h in range(HPG):
                    hp, hi = h // 2, h % 2
                    for kt2 in range(first, qt + 1):
                        j = qt - kt2
                        nc.tensor.matmul(
                            o_ps[:, h, :],
                            pT_map[(kt2, hp)][:, hi, j * 128 : (j + 1) * 128],
                            v_sb[:, kt2, h * VC : (h + 1) * VC],
                            start=(kt2 == first), stop=(kt2 == qt),
                        )
                if qt >= 2:
                    for hp in range(2):
                        del pT_map[(qt - 2, hp)]

            def emit_o_nrm(qt):
                o_ps = o_map.pop(qt)
                rec4 = stats.tile([128, HPG], FP, tag="rec4", name="rec4", bufs=3)
                nc.vector.reciprocal(rec4, _ap(o_ps, D, [[VC, HPG]]))
                o_nrm = work.tile([128, HPG * D], BF, tag="o_nrm", name="o_nrm", bufs=4)
                nc.vector.tensor_mul(
                    o_nrm.rearrange("p (h d) -> p h d", h=HPG),
                    _ap(o_ps, 0, [[VC, HPG], [1, D]]),
                    _ap(rec4, 0, [[1, HPG], [0, D]]),
                )
                onrm_map[qt] = o_nrm

            # ---- S6: transpose o into hoT (PE + DVE) ----------------------
            def emit_oT(qt, r):
                o_nrm = onrm_map.pop(qt)
                tps = get_tps(r)
                for hp in range(2):
                    nc.tensor.transpose(
                        tps[:, 512 + hp * 128 : 512 + (hp + 1) * 128],
                        o_nrm[:, hp * 128 : (hp + 1) * 128], idb_sb,
                    )

            def emit_oT_copy(qt):
                tps = tps_map["cur"]
                g, r = qt // 4, qt % 4
                dst = hoT_sb[g][:, :, r * 128 : (r + 1) * 128]
                nc.vector.tensor_copy(
                    dst, tps[:, 512:768].rearrange("p (a b) -> p a b", a=2)
                )

            # ---- S3/S7: out-projection halves (PE) + evac (ACT/DVE) -------
            def emit_c_mm(m, half):
                g, r = m // 4, m % 4
                rsl = slice(r * 128, (r + 1) * 128)
                nsl = slice(half * 512, (half + 1) * 512)
                # late tiles run after the qk projections have drained; their
                # half-0 borrows the (identically shaped) qk banks so the two
                # halves never serialize through the single c bank
                if m >= 8 and half == 0:
                    c_ps = ps.tile([128, 512], FP, tag="qk_ps", bufs=2, name="c_ps")
                elif m >= 12 and half == 1:
                    # the scores banks are idle once the last exp has run
                    c_ps = ps.tile([128, 512], FP, tag="s_ps", bufs=2, name="c_ps")
                else:
                    c_ps = ps.tile([128, 512], FP, tag="c_ps", bufs=1, name="c_ps")
                c_map[(m, half)] = c_ps
                for c in range(2):
                    nc.tensor.matmul(
                        c_ps, hoT_sb[g][:, c, rsl], wo_sb[:, c, nsl],
                        start=(c == 0), stop=(c == 1),
                    )

            def emit_c_copy(m, half):
                c_ps = c_map.pop((m, half))
                if half == 0:
                    osb_map[m] = outst.tile([128, E], BF, tag="o_sb", name="o_sb")
                    nc.scalar.copy(osb_map[m][:, 0:512], c_ps)
                else:
                    nc.vector.tensor_copy(osb_map[m][:, 512:1024], c_ps)

            def emit_out_dma(m, half=None):
                if half == 0:
                    nc.sync.dma_start(
                        out=out[m * 128 : (m + 1) * 128, 0:512],
                        in_=osb_map[m][:, 0:512],
                    )
                    return
                o_sb = osb_map.pop(m)
                if half == 1:
                    nc.sync.dma_start(
                        out=out[m * 128 : (m + 1) * 128, 512:1024],
                        in_=o_sb[:, 512:1024],
                    )
                else:
                    nc.sync.dma_start(out=out[m * 128 : (m + 1) * 128, :], in_=o_sb)

            # ---- pipelined rounds -----------------------------------------
            # Every producer->consumer pair gets >= 1 full round of slack so
            # the list scheduler never has a zero-slack cross-engine hop:
            # stats chain for tile m spans rounds m..m+1, aT(m) at m+2,
            # scores kt=r-5, o qt=r-6, oT qt=r-7, outproj m=r-9. After the
            # projections drain (last 3 tiles of each stage) the engines are
            # idle, so those run with minimal dependency-limited lags.
            def r_scores(kt):
                return kt + 5 if kt <= NT - 4 else kt + 4

            def r_o(qt):
                return qt + 6 if qt <= NT - 4 else qt + 5

            def r_oT(qt):
                return qt + 7 if qt <= NT - 4 else qt + 6

            def r_c(m):
                return m + 9 if m <= NT - 4 else m + 7

            NR = NT + 7
            for r in range(NR):
                kts = [kt for kt in range(NT) if r_scores(kt) == r]
                qos = [qt for qt in range(NT) if r_o(qt) == r]
                qTs = [qt for qt in range(NT) if r_oT(qt) == r]
                mcs = [m for m in range(NT) if r_c(m) == r]
                for qt in qTs:
                    emit_o_nrm(qt)          # DVE: o_ps from an earlier round
                if 2 <= r <= NT + 1:
                    emit_aT(r - 2, r)       # PE: q/k transposes
                    emit_aT_copy(r - 2)     # DVE + ACT
                for kt in kts:
                    s1 = emit_b_mm(kt, 0, 0)           # PE: scores h0
                    emit_b_exp(kt, 0, 0, s1)           # ACT
                    s2 = emit_b_mm(kt, 0, 1)           # PE: scores h1
                    emit_b_exp(kt, 0, 1, s2)           # ACT
                    emit_b_sel(kt, 0)                  # Pool
                if r < NT:
                    emit_qk_mm(r)           # PE: qk projection
                    emit_sq(r)              # ACT
                    emit_reduce(r)          # DVE
                if r == 1:
                    nc.sync.dma_start(out=xmap["1a"], in_=xT_r[:, :, 512:768])
                    nc.sync.dma_start(out=xmap["1b"], in_=xT_r[:, :, 768:1024])
                    nc.sync.dma_start(
                        out=wo_sb, in_=wo.rearrange("(k p) e -> p k e", p=128)
                    )
                for m in mcs:
                    emit_c_mm(m, 0)         # PE: outproj half 0
                    emit_c_copy(m, 0)       # ACT
                    if m >= NT - 3:
                        emit_out_dma(m, 0)  # tail tiles: ship half 0 early
                for kt in kts:
                    s3 = emit_b_mm(kt, 1, 0)           # PE: scores h2
                    emit_b_exp(kt, 1, 0, s3)           # ACT
                    s4 = emit_b_mm(kt, 1, 1)           # PE: scores h3
                    emit_b_exp(kt, 1, 1, s4)           # ACT
                    emit_b_sel(kt, 1)                  # Pool
                if 2 <= r <= NT + 1:
                    emit_v_mm(r - 2)        # PE: v projection (lagged 2)
                    emit_v_copy(r - 2)      # DVE
                if r < NT:
                    rstd8 = emit_rstd(r)    # ACT ln+exp
                    emit_nrm(r, rstd8)      # DVE
                if r in (5, 9):
                    load_x((r - 1) // 4 + 1)  # prefetch x chunks 2, 3
                for qt in qTs:
                    emit_oT(qt, r)          # PE: o transposes (same tps tile)
                    emit_oT_copy(qt)        # DVE
                for m in mcs:
                    emit_c_mm(m, 1)         # PE: outproj half 1
                    emit_c_copy(m, 1)       # DVE
                    emit_out_dma(m, 1 if m >= NT - 3 else None)  # SP dma
                for qt in qos:
                    emit_o(qt)              # PE: o blob (last: most margin)

    nc.compile()
    return nc


def _host_inputs(x, wq, wk, wv, wo, qn_w, kn_w):
    """Build the 8 per-core input maps."""
    qn2 = (np.tile(qn_w, 2) * 0.125).astype(np.float32)[:, None]
    kn2 = np.tile(kn_w, 2).astype(np.float32)[:, None]

    bf = ml_dtypes.bfloat16
    xT = np.ascontiguousarray(np.transpose(x, (0, 2, 1))).astype(bf)  # [B, E, T]
    in_maps = []
    for core in range(NCORES):
        b, g = divmod(core, HG)
        cs = slice(g * COLS, (g + 1) * COLS)
        wqk = np.concatenate([wq[:, cs], wk[:, cs]], axis=1).astype(bf)
        in_maps.append(
            {
                "xT": xT[b],
                "wqk": np.ascontiguousarray(wqk),
                "wv": np.ascontiguousarray(wv[:, cs]).astype(bf),
                "wo": np.ascontiguousarray(wo[cs, :]).astype(bf),
                "qn2": qn2,
                "kn2": kn2,
            }
        )
    return in_maps


def run(trace=False, **inputs):
    if "nc" not in _cache:
        _cache["nc"] = _build()
    nc = _cache["nc"]
    in_maps = _host_inputs(
        np.asarray(inputs["x"]), np.asarray(inputs["wq"]), np.asarray(inputs["wk"]),
        np.asarray(inputs["wv"]), np.asarray(inputs["wo"]),
        np.asarray(inputs["qn_w"]), np.asarray(inputs["kn_w"]),
    )
    res = bass_utils.run_bass_kernel_spmd(
        nc, in_maps, core_ids=list(range(NCORES)), trace=trace
    )
    bo = np.asarray(inputs["bo"], np.float32)
    outs = []
    for b in range(B):
        acc = np.zeros((T, E), np.float32)
        for g in range(HG):
            acc += np.asarray(res.results[b * HG + g]["out"], np.float32)
        outs.append(acc + bo[None, :])
    return np.stack(outs), res


def kernel(**inputs):
    out, _ = run(trace=False, **inputs)
    return out
